# revision 7
# baseline (speedup 1.0000x reference)
"""Multi-head attention (QKV proj + RMS-norm + RoPE + softmax + out-proj) on 8 trn2 cores.

Sharding: core c handles batch c//4 and heads 4*(c%4) .. 4*(c%4)+3.
Each core returns a partial [S, D] output (sum over its 4 heads); the host
sums the 4 partials per batch and adds b_O.

Device kernel layout choices:
  - Q/K kept transposed [F=128, S] on chip; scores computed transposed [q, p]
    so softmax denominators come from a partition reduction (PE ones-matmul)
    and no big transposes are ever needed.
  - All heavy matmuls run in float32r (full PE rate, ~2e-4 rel err).
  - RMS-norm weight w is folded into the RoPE constants on the host:
    cos_w = cos.T * w[:, None] and R_w = (R @ diag(w)).T, using that sin/cos
    tables are 64-periodic along the feature axis.
  - eps of the rms denominator is folded into Sqrt's bias: sqrt(ms + 2e-6)
    ~= rms + 1e-6 to within ~1e-12.
"""

import numpy as np

import concourse.bacc as bacc
import concourse.mybir as mybir
import concourse.tile as tile
from concourse.bass_utils import run_bass_kernel_spmd

F32 = mybir.dt.float32
F32R = mybir.dt.float32r
AF = mybir.ActivationFunctionType

B, S, D, H, F = 2, 2048, 2048, 16, 128
HC = 4           # heads per core
NCORES = 8
PT = 128         # partition tile
CW = 512         # column chunk width
EPS = 1e-6


def build_program(S=S, D=D, HC=HC, F=F):
    nd = D // PT          # contraction tiles
    nq = S // PT          # q tiles (keys) / p tiles (queries)
    nch = S // CW         # 512-wide chunks of s
    nmch = D // CW        # 512-wide chunks of model dim
    assert nq % 4 == 0

    nc = bacc.Bacc("TRN2", target_bir_lowering=False, debug=True)

    xqt = nc.declare_dram_parameter("xqt", [D, S], F32, isOutput=False)
    xkt = nc.declare_dram_parameter("xkt", [D, S], F32, isOutput=False)
    xvt = nc.declare_dram_parameter("xvt", [D, S], F32, isOutput=False)
    wq = nc.declare_dram_parameter("wq", [HC, D, F], F32, isOutput=False)
    wk = nc.declare_dram_parameter("wk", [HC, D, F], F32, isOutput=False)
    wv = nc.declare_dram_parameter("wv", [D, HC * F], F32, isOutput=False)
    wo = nc.declare_dram_parameter("wo", [HC, F, D], F32, isOutput=False)
    cosq = nc.declare_dram_parameter("cosq", [F, S], F32, isOutput=False)
    cosk = nc.declare_dram_parameter("cosk", [F, S], F32, isOutput=False)
    sint = nc.declare_dram_parameter("sint", [F, S], F32, isOutput=False)
    rwq = nc.declare_dram_parameter("rwq", [F, F], F32, isOutput=False)
    onesd = nc.declare_dram_parameter("ones", [PT, PT], F32, isOutput=False)
    rwk = nc.declare_dram_parameter("rwk", [F, F], F32, isOutput=False)
    outp = nc.declare_dram_parameter("out", [S, D], F32, isOutput=True)

    with tile.TileContext(nc) as tc:
        with tc.tile_pool(name="outer", bufs=1) as outer:
            ones_col = outer.tile([PT, 1], F32R)
            nc.sync.dma_start(out=ones_col[:], in_=onesd[:, 0:1].bitcast(F32R))
            ones_row = outer.tile([1, PT], F32R)
            nc.sync.dma_start(out=ones_row[:], in_=onesd[0:1, :].bitcast(F32R))
            eps_sb = outer.tile([1, 1], F32)
            nc.vector.memset(eps_sb[:], 2 * EPS)
            rq_sb = outer.tile([F, F], F32R, tag="rq")
            nc.sync.dma_start(out=rq_sb[:], in_=rwq[:].bitcast(F32R))
            rk_sb = outer.tile([F, F], F32R, tag="rk")
            nc.sync.dma_start(out=rk_sb[:], in_=rwk[:].bitcast(F32R))

            qt_sb = [outer.tile([F, S], F32R, tag=f"qt{h}", name=f"qt{h}") for h in range(HC)]
            kt_sb = [outer.tile([F, S], F32R, tag=f"kt{h}", name=f"kt{h}") for h in range(HC)]
            v_sb = outer.tile([PT, nq, HC * F], F32R, tag="vnat")
            at_sb = [outer.tile([F, S], F32R, tag=f"at{h}", name=f"at{h}") for h in range(HC)]

            # ---------------- Q/K projections + rms + rope ----------------
            with (
                tc.tile_pool(name="accps", bufs=HC, space="PSUM") as accps,
                tc.tile_pool(name="proj", bufs=1) as proj,
                tc.tile_pool(name="qkps", bufs=1, space="PSUM") as qkps,
            ):
                for xdram, wdram, cosdram, r_sb, outs in (
                    (xqt, wq, cosq, rq_sb, qt_sb),
                    (xkt, wk, cosk, rk_sb, kt_sb),
                ):
                    w_tiles = []
                    for h in range(HC):
                        wt = proj.tile([PT, nd, F], F32R, tag=f"w{h}", name=f"w{h}")
                        nc.sync.dma_start(
                            out=wt[:],
                            in_=wdram[h]
                            .rearrange("(do dp) f -> dp do f", dp=PT)
                            .bitcast(F32R),
                        )
                        w_tiles.append(wt)
                    for c in range(nch):
                        cs = slice(c * CW, (c + 1) * CW)
                        sin_c = proj.tile([F, CW], F32, tag="sin_c", bufs=2)
                        nc.sync.dma_start(out=sin_c[:], in_=sint[:, cs])
                        cos_c = proj.tile([F, CW], F32, tag="cos_c", bufs=2)
                        nc.sync.dma_start(out=cos_c[:], in_=cosdram[:, cs])
                        accs = [
                            accps.tile([PT, CW], F32, tag="acc", name="acc")
                            for _ in range(HC)
                        ]
                        for d in range(nd):
                            xt = proj.tile([PT, CW], F32R, tag="x", bufs=3)
                            nc.sync.dma_start(
                                out=xt[:],
                                in_=xdram[d * PT : (d + 1) * PT, cs].bitcast(F32R),
                            )
                            for h in range(HC):
                                nc.tensor.matmul(
                                    accs[h][:],
                                    w_tiles[h][:, d, :],
                                    xt[:],
                                    start=(d == 0),
                                    stop=(d == nd - 1),
                                )
                        for h in range(HC):
                            # rms: 1/(sqrt(mean(q^2)) + eps), folded eps
                            sq = proj.tile([PT, CW], F32R, tag="sq", bufs=2)
                            nc.scalar.activation(sq[:], accs[h][:], AF.Square)
                            ssum = qkps.tile([1, CW], F32, tag="ssum")
                            nc.tensor.matmul(
                                ssum[:], ones_col[:], sq[:], start=True, stop=True
                            )
                            rms = proj.tile([1, CW], F32, tag="rms", bufs=2)
                            nc.scalar.activation(
                                rms[:], ssum[:], AF.Sqrt, scale=1.0 / F,
                                bias=eps_sb[:],
                            )
                            rcp = proj.tile([1, CW], F32R, tag="rcp", bufs=2)
                            with nc.allow_low_precision("f32r rank-1 scale"):
                                nc.vector.reciprocal(rcp[:], rms[:])
                            bc = qkps.tile([PT, CW], F32, tag="bc")
                            nc.tensor.matmul(
                                bc[:], ones_row[:], rcp[:], start=True, stop=True
                            )
                            bcs = proj.tile([PT, CW], F32, tag="bcs", bufs=2)
                            nc.vector.tensor_copy(bcs[:], bc[:])
                            qr = proj.tile([PT, CW], F32R, tag="qr", bufs=2)
                            nc.vector.tensor_mul(qr[:], accs[h][:], bcs[:])
                            qs = proj.tile([PT, CW], F32R, tag="qs", bufs=2)
                            nc.vector.tensor_mul(qs[:], qr[:].bitcast(F32), sin_c[:])
                            rot = qkps.tile([F, CW], F32, tag="rot", bufs=2)
                            nc.tensor.matmul(
                                rot[:], r_sb[:], qs[:], start=True, stop=True
                            )
                            qc = proj.tile([PT, CW], F32R, tag="qc", bufs=2)
                            nc.vector.tensor_mul(qc[:], qr[:].bitcast(F32), cos_c[:])
                            nc.vector.tensor_add(
                                outs[h][:, cs], qc[:].bitcast(F32), rot[:]
                            )

            # ---------------- V projection (natural [q, f] layout) ----------------
            with (
                tc.tile_pool(name="projv", bufs=1) as pv,
                tc.tile_pool(name="vps", bufs=1, space="PSUM") as vps,
            ):
                HW = S // 2
                for half in range(2):
                    hs = slice(half * HW, (half + 1) * HW)
                    vaccs = [
                        vps.tile([PT, HC * F], F32, tag="vacc", bufs=nq // 2,
                                 name="vacc")
                        for _ in range(nq // 2)
                    ]
                    for d in range(nd):
                        xt = pv.tile([PT, HW], F32R, tag="xv", bufs=3)
                        nc.sync.dma_start(
                            out=xt[:],
                            in_=xvt[d * PT : (d + 1) * PT, hs].bitcast(F32R),
                        )
                        wvd = pv.tile([PT, HC * F], F32R, tag="wvd", bufs=3)
                        nc.sync.dma_start(
                            out=wvd[:],
                            in_=wv[d * PT : (d + 1) * PT, :].bitcast(F32R),
                        )
                        for j in range(nq // 2):
                            nc.tensor.matmul(
                                vaccs[j][:],
                                xt[:, j * PT : (j + 1) * PT],
                                wvd[:],
                                start=(d == 0),
                                stop=(d == nd - 1),
                            )
                    for j in range(nq // 2):
                        nc.vector.tensor_copy(
                            v_sb[:, half * (nq // 2) + j, :], vaccs[j][:]
                        )

            # ---------------- attention ----------------
            with (
                tc.tile_pool(name="attn", bufs=1) as attn,
                tc.tile_pool(name="aps", bufs=1, space="PSUM") as aps,
            ):
                scale = float(F) ** -0.5
                nqh = nq // 2  # q tiles per half
                for h in range(HC):
                    for c in range(nch):
                        cs = slice(c * CW, (c + 1) * CW)
                        pv_ps = aps.tile([F, CW], F32, tag="pv", bufs=2)
                        parts = []
                        for g in range(2):
                            e_sb = attn.tile([PT, nqh, CW], F32R, tag=f"e{g}")
                            for jp in range(nqh // 2):
                                st = aps.tile([PT, 2, CW], F32, tag="st", bufs=2)
                                for k in range(2):
                                    qt_i = g * nqh + jp * 2 + k
                                    nc.tensor.matmul(
                                        st[:, k, :],
                                        kt_sb[h][:, qt_i * PT : (qt_i + 1) * PT],
                                        qt_sb[h][:, cs],
                                        start=True,
                                        stop=True,
                                    )
                                nc.scalar.activation(
                                    e_sb[:, jp * 2 : jp * 2 + 2, :],
                                    st[:],
                                    AF.Exp,
                                    scale=scale,
                                )
                            for j in range(nqh):
                                qt_i = g * nqh + j
                                nc.tensor.matmul(
                                    pv_ps[:],
                                    v_sb[:, qt_i, h * F : (h + 1) * F],
                                    e_sb[:, j, :],
                                    start=(qt_i == 0),
                                    stop=(qt_i == nq - 1),
                                )
                            part = attn.tile([PT, CW], F32, tag=f"part{g}", bufs=2)
                            nc.vector.reduce_sum(
                                out=part[:],
                                in_=e_sb[:].bitcast(F32).rearrange("p a b -> p b a"),
                                axis=mybir.AxisListType.X,
                            )
                            parts.append(part)
                        esum = attn.tile([PT, CW], F32R, tag="esum", bufs=2)
                        nc.vector.tensor_add(esum[:], parts[0][:], parts[1][:])
                        dn = aps.tile([1, CW], F32, tag="dn", bufs=1)
                        nc.tensor.matmul(
                            dn[:], ones_col[:], esum[:], start=True, stop=True
                        )
                        rcd = attn.tile([1, CW], F32R, tag="rcd", bufs=2)
                        with nc.allow_low_precision("f32r softmax denom"):
                            nc.vector.reciprocal(rcd[:], dn[:])
                        rbc = aps.tile([PT, CW], F32, tag="rbc", bufs=1)
                        nc.tensor.matmul(
                            rbc[:], ones_row[:], rcd[:], start=True, stop=True
                        )
                        rbs = attn.tile([PT, CW], F32, tag="rbs", bufs=2)
                        nc.vector.tensor_copy(rbs[:], rbc[:])
                        nc.vector.tensor_mul(at_sb[h][:, cs], pv_ps[:], rbs[:])

            # ---------------- out projection ----------------
            with (
                tc.tile_pool(name="oproj", bufs=1) as op,
                tc.tile_pool(name="ops", bufs=1, space="PSUM") as ops,
            ):
                wo_sb = []
                for h in range(HC):
                    wt = op.tile([F, D], F32R, tag=f"wo{h}", name=f"wo{h}")
                    nc.sync.dma_start(out=wt[:], in_=wo[h].bitcast(F32R))
                    wo_sb.append(wt)
                for t in range(nq):
                    po = ops.tile([PT, D], F32, tag="po", bufs=2)
                    for h in range(HC):
                        for m in range(nmch):
                            nc.tensor.matmul(
                                po[:, m * CW : (m + 1) * CW],
                                at_sb[h][:, t * PT : (t + 1) * PT],
                                wo_sb[h][:, m * CW : (m + 1) * CW],
                                start=(h == 0),
                                stop=(h == HC - 1),
                            )
                    so = op.tile([PT, D], F32, tag="so", bufs=2)
                    nc.vector.tensor_copy(so[:], po[:])
                    nc.sync.dma_start(out=outp[t * PT : (t + 1) * PT, :], in_=so[:])

    nc.finalize()
    return nc


def make_rope_consts(S_, F_, w):
    """cos/sin tables transposed to [F, S]; cos gets the norm weight folded in."""
    half = F_ // 2
    freq = 1.0 / (10000.0 ** (np.arange(half, dtype=np.float32) / half))
    t = np.arange(S_, dtype=np.float32)
    freqs = t[:, None] * freq[None, :]
    freqs = np.concatenate([freqs, freqs], -1)  # [S, F]
    cosT = np.cos(freqs).T.astype(np.float32)   # [F, S]
    sinT = np.sin(freqs).T.astype(np.float32)
    cos_w = (cosT * w[:, None]).astype(np.float32)
    return np.ascontiguousarray(cos_w), np.ascontiguousarray(sinT)


def make_rot_lhsT(F_, w):
    """lhsT of the rotate-half matmul with the norm weight folded in."""
    half = F_ // 2
    R = np.zeros((F_, F_), np.float32)
    for f in range(half):
        R[f, f + half] = -1.0
    for f in range(half, F_):
        R[f, f - half] = 1.0
    return np.ascontiguousarray((R @ np.diag(w)).T.astype(np.float32))


_ONES = np.ones((PT, PT), np.float32)

# test harness hooks: set _RUN_KWARGS = {"trace": True} before calling kernel()
# to capture an NTFF profile; the BassKernelResults lands in _LAST_RESULTS.
_RUN_KWARGS = {}
_LAST_RESULTS = None

_PROGRAM = None


def _get_program():
    global _PROGRAM
    if _PROGRAM is None:
        _PROGRAM = build_program()
    return _PROGRAM


def kernel(
    query_input,
    key_input,
    value_input,
    W_Q,
    W_K,
    W_V,
    W_O,
    b_Q,
    b_K,
    b_V,
    b_O,
    q_norm_w,
    k_norm_w,
):
    query_input = np.asarray(query_input, np.float32)
    key_input = np.asarray(key_input, np.float32)
    value_input = np.asarray(value_input, np.float32)
    W_Q = np.asarray(W_Q, np.float32)
    W_K = np.asarray(W_K, np.float32)
    W_V = np.asarray(W_V, np.float32)
    W_O = np.asarray(W_O, np.float32)
    q_norm_w = np.asarray(q_norm_w, np.float32)
    k_norm_w = np.asarray(k_norm_w, np.float32)

    nc = _get_program()

    cos_wq, sinT = make_rope_consts(S, F, q_norm_w)
    cos_wk, _ = make_rope_consts(S, F, k_norm_w)
    rwq = make_rot_lhsT(F, q_norm_w)
    rwk = make_rot_lhsT(F, k_norm_w)

    in_maps = []
    for c in range(NCORES):
        b = c // (NCORES // B)
        hs = (c % (NCORES // B)) * HC
        in_maps.append(
            {
                "xqt": np.ascontiguousarray(query_input[b].T),
                "xkt": np.ascontiguousarray(key_input[b].T),
                "xvt": np.ascontiguousarray(value_input[b].T),
                "wq": np.ascontiguousarray(W_Q[hs : hs + HC]),
                "wk": np.ascontiguousarray(W_K[hs : hs + HC]),
                "wv": np.ascontiguousarray(
                    W_V[hs : hs + HC].transpose(1, 0, 2).reshape(D, HC * F)
                ),
                "wo": np.ascontiguousarray(W_O[hs : hs + HC]),
                "ones": _ONES,
                "cosq": cos_wq,
                "cosk": cos_wk,
                "sint": sinT,
                "rwq": rwq,
                "rwk": rwk,
            }
        )

    res = run_bass_kernel_spmd(
        nc, in_maps, core_ids=list(range(NCORES)), **_RUN_KWARGS
    )
    global _LAST_RESULTS
    _LAST_RESULTS = res
    out = np.zeros((B, S, D), np.float32)
    for c in range(NCORES):
        out[c // (NCORES // B)] += res.results[c]["out"]
    out += np.asarray(b_O, np.float32)[None, None, :]
    return out


# revision 9
# speedup vs baseline: 1.1301x; 1.1301x over previous
"""Multi-head attention (QKV proj + RMS-norm + RoPE + softmax + out-proj) on 8 trn2 cores.

Sharding: core c handles batch c//4 and heads 4*(c%4) .. 4*(c%4)+3.
Each core returns a partial [S, D] output (sum over its 4 heads); the host
sums the 4 partials per batch and adds b_O.

Device kernel layout choices:
  - Q/K kept transposed [F=128, S] on chip; scores computed transposed [q, p]
    so softmax denominators come from a partition reduction (PE ones-matmul)
    and no big transposes are ever needed.
  - All heavy matmuls run in float32r (full PE rate, ~2e-4 rel err).
  - RMS-norm weight w is folded into the RoPE constants on the host:
    cos_w = cos.T * w[:, None] and R_w = (R @ diag(w)).T, using that sin/cos
    tables are 64-periodic along the feature axis.
  - eps of the rms denominator is folded into Sqrt's bias: sqrt(ms + 2e-6)
    ~= rms + 1e-6 to within ~1e-12.
"""

import numpy as np

import concourse.bacc as bacc
import concourse.mybir as mybir
import concourse.tile as tile
from concourse.bass_utils import run_bass_kernel_spmd

F32 = mybir.dt.float32
F32R = mybir.dt.float32r
AF = mybir.ActivationFunctionType

B, S, D, H, F = 2, 2048, 2048, 16, 128
HC = 4           # heads per core
NCORES = 8
PT = 128         # partition tile
CW = 512         # column chunk width
EPS = 1e-6


def build_program(S=S, D=D, HC=HC, F=F):
    nd = D // PT          # contraction tiles
    nq = S // PT          # q tiles (keys) / p tiles (queries)
    nch = S // CW         # 512-wide chunks of s
    nmch = D // CW        # 512-wide chunks of model dim
    assert nq % 4 == 0

    nc = bacc.Bacc("TRN2", target_bir_lowering=False, debug=True)

    xqt = nc.declare_dram_parameter("xqt", [D, S], F32, isOutput=False)
    xkt = nc.declare_dram_parameter("xkt", [D, S], F32, isOutput=False)
    xvt = nc.declare_dram_parameter("xvt", [D, S], F32, isOutput=False)
    wq = nc.declare_dram_parameter("wq", [HC, D, F], F32, isOutput=False)
    wk = nc.declare_dram_parameter("wk", [HC, D, F], F32, isOutput=False)
    wv = nc.declare_dram_parameter("wv", [D, HC * F], F32, isOutput=False)
    wo = nc.declare_dram_parameter("wo", [HC, F, D], F32, isOutput=False)
    cosq = nc.declare_dram_parameter("cosq", [F, S], F32, isOutput=False)
    cosk = nc.declare_dram_parameter("cosk", [F, S], F32, isOutput=False)
    sint = nc.declare_dram_parameter("sint", [F, S], F32, isOutput=False)
    rwq = nc.declare_dram_parameter("rwq", [F, F], F32, isOutput=False)
    onesd = nc.declare_dram_parameter("ones", [PT, PT], F32, isOutput=False)
    rwk = nc.declare_dram_parameter("rwk", [F, F], F32, isOutput=False)
    outp = nc.declare_dram_parameter("out", [S, D], F32, isOutput=True)

    with tile.TileContext(nc) as tc:
        with tc.tile_pool(name="outer", bufs=1) as outer:
            ones_col = outer.tile([PT, 1], F32R)
            nc.sync.dma_start(out=ones_col[:], in_=onesd[:, 0:1].bitcast(F32R))
            ones_row = outer.tile([1, PT], F32R)
            nc.sync.dma_start(out=ones_row[:], in_=onesd[0:1, :].bitcast(F32R))
            eps_sb = outer.tile([1, 1], F32)
            nc.vector.memset(eps_sb[:], 2 * EPS)
            ones_row32 = outer.tile([1, PT], F32)
            nc.vector.memset(ones_row32[:], 1.0)
            rq_sb = outer.tile([F, F], F32R, tag="rq")
            nc.sync.dma_start(out=rq_sb[:], in_=rwq[:].bitcast(F32R))
            rk_sb = outer.tile([F, F], F32R, tag="rk")
            nc.sync.dma_start(out=rk_sb[:], in_=rwk[:].bitcast(F32R))

            qt_sb = [outer.tile([F, S], F32R, tag=f"qt{h}", name=f"qt{h}") for h in range(HC)]
            kt_sb = [outer.tile([F, S], F32R, tag=f"kt{h}", name=f"kt{h}") for h in range(HC)]
            v_sb = outer.tile([PT, nq, HC * F], F32R, tag="vnat")
            at_sb = [outer.tile([F, S], F32R, tag=f"at{h}", name=f"at{h}") for h in range(HC)]

            # ---------------- Q/K projections + rms + rope ----------------
            with (
                tc.tile_pool(name="accps", bufs=HC, space="PSUM") as accps,
                tc.tile_pool(name="proj", bufs=1) as proj,
                tc.tile_pool(name="qkps", bufs=1, space="PSUM") as qkps,
            ):
                for xdram, wdram, cosdram, r_sb, outs in (
                    (xqt, wq, cosq, rq_sb, qt_sb),
                    (xkt, wk, cosk, rk_sb, kt_sb),
                ):
                    w_tiles = []
                    for h in range(HC):
                        wt = proj.tile([PT, nd, F], F32R, tag=f"w{h}", name=f"w{h}")
                        nc.sync.dma_start(
                            out=wt[:],
                            in_=wdram[h]
                            .rearrange("(do dp) f -> dp do f", dp=PT)
                            .bitcast(F32R),
                        )
                        w_tiles.append(wt)
                    for c in range(nch):
                        cs = slice(c * CW, (c + 1) * CW)
                        sin_c = proj.tile([F, CW], F32, tag="sin_c", bufs=2)
                        nc.sync.dma_start(out=sin_c[:], in_=sint[:, cs])
                        cos_c = proj.tile([F, CW], F32, tag="cos_c", bufs=2)
                        nc.sync.dma_start(out=cos_c[:], in_=cosdram[:, cs])
                        accs = [
                            accps.tile([PT, CW], F32, tag="acc", name="acc")
                            for _ in range(HC)
                        ]
                        for d in range(nd):
                            xt = proj.tile([PT, CW], F32R, tag="x", bufs=3)
                            nc.sync.dma_start(
                                out=xt[:],
                                in_=xdram[d * PT : (d + 1) * PT, cs].bitcast(F32R),
                            )
                            for h in range(HC):
                                nc.tensor.matmul(
                                    accs[h][:],
                                    w_tiles[h][:, d, :],
                                    xt[:],
                                    start=(d == 0),
                                    stop=(d == nd - 1),
                                )
                        for h in range(HC):
                            # rms: 1/(sqrt(mean(q^2)) + eps), folded eps
                            sq = proj.tile([PT, CW], F32R, tag="sq", bufs=2)
                            nc.scalar.activation(sq[:], accs[h][:], AF.Square)
                            ssum = qkps.tile([1, CW], F32, tag="ssum")
                            nc.tensor.matmul(
                                ssum[:], ones_col[:], sq[:], start=True, stop=True
                            )
                            rms = proj.tile([1, CW], F32, tag="rms", bufs=2)
                            nc.scalar.activation(
                                rms[:], ssum[:], AF.Sqrt, scale=1.0 / F,
                                bias=eps_sb[:],
                            )
                            rcp = proj.tile([1, CW], F32, tag="rcp", bufs=2)
                            nc.vector.reciprocal_approx_fast(rcp[:], rms[:])
                            bc = qkps.tile([PT, CW], F32, tag="bc")
                            nc.tensor.matmul(
                                bc[:], ones_row32[:], rcp[:], start=True, stop=True
                            )
                            qs = proj.tile([PT, CW], F32R, tag="qs", bufs=2)
                            nc.vector.tensor_mul(qs[:], accs[h][:], sin_c[:])
                            rot = qkps.tile([F, CW], F32, tag="rot", bufs=2)
                            nc.tensor.matmul(
                                rot[:], r_sb[:], qs[:], start=True, stop=True
                            )
                            qc = proj.tile([PT, CW], F32, tag="qc", bufs=2)
                            nc.vector.tensor_mul(qc[:], accs[h][:], cos_c[:])
                            qu = proj.tile([PT, CW], F32, tag="qu", bufs=2)
                            nc.vector.tensor_add(qu[:], qc[:], rot[:])
                            nc.vector.tensor_mul(outs[h][:, cs], qu[:], bc[:])

            # ---------------- V projection (natural [q, f] layout) ----------------
            with (
                tc.tile_pool(name="projv", bufs=1) as pv,
                tc.tile_pool(name="vps", bufs=1, space="PSUM") as vps,
            ):
                HW = S // 2
                for half in range(2):
                    hs = slice(half * HW, (half + 1) * HW)
                    vaccs = [
                        vps.tile([PT, HC * F], F32, tag="vacc", bufs=nq // 2,
                                 name="vacc")
                        for _ in range(nq // 2)
                    ]
                    for d in range(nd):
                        xt = pv.tile([PT, HW], F32R, tag="xv", bufs=3)
                        nc.sync.dma_start(
                            out=xt[:],
                            in_=xvt[d * PT : (d + 1) * PT, hs].bitcast(F32R),
                        )
                        wvd = pv.tile([PT, HC * F], F32R, tag="wvd", bufs=3)
                        nc.sync.dma_start(
                            out=wvd[:],
                            in_=wv[d * PT : (d + 1) * PT, :].bitcast(F32R),
                        )
                        for j in range(nq // 2):
                            nc.tensor.matmul(
                                vaccs[j][:],
                                xt[:, j * PT : (j + 1) * PT],
                                wvd[:],
                                start=(d == 0),
                                stop=(d == nd - 1),
                            )
                    for j in range(nq // 2):
                        nc.vector.tensor_copy(
                            v_sb[:, half * (nq // 2) + j, :], vaccs[j][:]
                        )

            # ---------------- attention ----------------
            with (
                tc.tile_pool(name="attn", bufs=1) as attn,
                tc.tile_pool(name="aps", bufs=1, space="PSUM") as aps,
            ):
                scale = float(F) ** -0.5
                nqh = nq // 2  # q tiles per half
                for h in range(HC):
                    for c in range(nch):
                        cs = slice(c * CW, (c + 1) * CW)
                        pv_ps = aps.tile([F, CW], F32, tag="pv", bufs=2)
                        dn = aps.tile([1, CW], F32, tag="dn", bufs=1)
                        for g in range(2):
                            e_sb = attn.tile([PT, nqh, CW], F32R, tag=f"e{g}")
                            for jp in range(nqh // 2):
                                st = aps.tile([PT, 2, CW], F32, tag="st", bufs=2)
                                for k in range(2):
                                    qt_i = g * nqh + jp * 2 + k
                                    nc.tensor.matmul(
                                        st[:, k, :],
                                        kt_sb[h][:, qt_i * PT : (qt_i + 1) * PT],
                                        qt_sb[h][:, cs],
                                        start=True,
                                        stop=True,
                                    )
                                nc.scalar.activation(
                                    e_sb[:, jp * 2 : jp * 2 + 2, :],
                                    st[:],
                                    AF.Exp,
                                    scale=scale,
                                )
                            for j in range(nqh):
                                qt_i = g * nqh + j
                                nc.tensor.matmul(
                                    pv_ps[:],
                                    v_sb[:, qt_i, h * F : (h + 1) * F],
                                    e_sb[:, j, :],
                                    start=(qt_i == 0),
                                    stop=(qt_i == nq - 1),
                                )
                                nc.tensor.matmul(
                                    dn[:],
                                    ones_col[:],
                                    e_sb[:, j, :],
                                    start=(qt_i == 0),
                                    stop=(qt_i == nq - 1),
                                )
                        rcd = attn.tile([1, CW], F32, tag="rcd", bufs=2)
                        nc.vector.reciprocal_approx_fast(rcd[:], dn[:])
                        rbc = aps.tile([PT, CW], F32, tag="rbc", bufs=1)
                        nc.tensor.matmul(
                            rbc[:], ones_row32[:], rcd[:], start=True, stop=True
                        )
                        rbs = attn.tile([PT, CW], F32, tag="rbs", bufs=2)
                        nc.scalar.copy(rbs[:], rbc[:])
                        nc.vector.tensor_mul(at_sb[h][:, cs], pv_ps[:], rbs[:])

            # ---------------- out projection ----------------
            with (
                tc.tile_pool(name="oproj", bufs=1) as op,
                tc.tile_pool(name="ops", bufs=1, space="PSUM") as ops,
            ):
                wo_sb = []
                for h in range(HC):
                    wt = op.tile([F, D], F32R, tag=f"wo{h}", name=f"wo{h}")
                    nc.sync.dma_start(out=wt[:], in_=wo[h].bitcast(F32R))
                    wo_sb.append(wt)
                for t in range(nq):
                    po = ops.tile([PT, D], F32, tag="po", bufs=2)
                    for h in range(HC):
                        for m in range(nmch):
                            nc.tensor.matmul(
                                po[:, m * CW : (m + 1) * CW],
                                at_sb[h][:, t * PT : (t + 1) * PT],
                                wo_sb[h][:, m * CW : (m + 1) * CW],
                                start=(h == 0),
                                stop=(h == HC - 1),
                            )
                    so = op.tile([PT, D], F32, tag="so", bufs=2)
                    nc.vector.tensor_copy(so[:], po[:])
                    nc.sync.dma_start(out=outp[t * PT : (t + 1) * PT, :], in_=so[:])

    nc.finalize()
    return nc


def make_rope_consts(S_, F_, w):
    """cos/sin tables transposed to [F, S]; cos gets the norm weight folded in."""
    half = F_ // 2
    freq = 1.0 / (10000.0 ** (np.arange(half, dtype=np.float32) / half))
    t = np.arange(S_, dtype=np.float32)
    freqs = t[:, None] * freq[None, :]
    freqs = np.concatenate([freqs, freqs], -1)  # [S, F]
    cosT = np.cos(freqs).T.astype(np.float32)   # [F, S]
    sinT = np.sin(freqs).T.astype(np.float32)
    cos_w = (cosT * w[:, None]).astype(np.float32)
    return np.ascontiguousarray(cos_w), np.ascontiguousarray(sinT)


def make_rot_lhsT(F_, w):
    """lhsT of the rotate-half matmul with the norm weight folded in."""
    half = F_ // 2
    R = np.zeros((F_, F_), np.float32)
    for f in range(half):
        R[f, f + half] = -1.0
    for f in range(half, F_):
        R[f, f - half] = 1.0
    return np.ascontiguousarray((R @ np.diag(w)).T.astype(np.float32))


_ONES = np.ones((PT, PT), np.float32)

# test harness hooks: set _RUN_KWARGS = {"trace": True} before calling kernel()
# to capture an NTFF profile; the BassKernelResults lands in _LAST_RESULTS.
_RUN_KWARGS = {}
_LAST_RESULTS = None

_PROGRAM = None


def _get_program():
    global _PROGRAM
    if _PROGRAM is None:
        _PROGRAM = build_program()
    return _PROGRAM


def kernel(
    query_input,
    key_input,
    value_input,
    W_Q,
    W_K,
    W_V,
    W_O,
    b_Q,
    b_K,
    b_V,
    b_O,
    q_norm_w,
    k_norm_w,
):
    query_input = np.asarray(query_input, np.float32)
    key_input = np.asarray(key_input, np.float32)
    value_input = np.asarray(value_input, np.float32)
    W_Q = np.asarray(W_Q, np.float32)
    W_K = np.asarray(W_K, np.float32)
    W_V = np.asarray(W_V, np.float32)
    W_O = np.asarray(W_O, np.float32)
    q_norm_w = np.asarray(q_norm_w, np.float32)
    k_norm_w = np.asarray(k_norm_w, np.float32)

    nc = _get_program()

    cos_wq, sinT = make_rope_consts(S, F, q_norm_w)
    cos_wk, _ = make_rope_consts(S, F, k_norm_w)
    rwq = make_rot_lhsT(F, q_norm_w)
    rwk = make_rot_lhsT(F, k_norm_w)

    in_maps = []
    for c in range(NCORES):
        b = c // (NCORES // B)
        hs = (c % (NCORES // B)) * HC
        in_maps.append(
            {
                "xqt": np.ascontiguousarray(query_input[b].T),
                "xkt": np.ascontiguousarray(key_input[b].T),
                "xvt": np.ascontiguousarray(value_input[b].T),
                "wq": np.ascontiguousarray(W_Q[hs : hs + HC]),
                "wk": np.ascontiguousarray(W_K[hs : hs + HC]),
                "wv": np.ascontiguousarray(
                    W_V[hs : hs + HC].transpose(1, 0, 2).reshape(D, HC * F)
                ),
                "wo": np.ascontiguousarray(W_O[hs : hs + HC]),
                "ones": _ONES,
                "cosq": cos_wq,
                "cosk": cos_wk,
                "sint": sinT,
                "rwq": rwq,
                "rwk": rwk,
            }
        )

    res = run_bass_kernel_spmd(
        nc, in_maps, core_ids=list(range(NCORES)), **_RUN_KWARGS
    )
    global _LAST_RESULTS
    _LAST_RESULTS = res
    out = np.zeros((B, S, D), np.float32)
    for c in range(NCORES):
        out[c // (NCORES // B)] += res.results[c]["out"]
    out += np.asarray(b_O, np.float32)[None, None, :]
    return out
